# revision 12
# baseline (speedup 1.0000x reference)
"""CapsuleTransformConv on 8 Trainium2 NeuronCores (fp16 pipeline).

Problem:  x [4,16,16,32,16] f32, matrix [288,16,512] f32.
          im2col (K=3, VALID) -> tile [4,14,14,288,16]
          votes  = einsum('bhwna,nac->bhwnc', tile, matrix)
          out    = votes.reshape(4,14,14,288,32,16)

Sharding: tensor-parallel over the filter*atom output axis (512 -> 64 per
core).  Every core reads the full x (2 MB) and its 64-wide weight slice;
writes its 784 x 288 x 64 output slice.

v2 (fp16) design, from the v1 (f32r) trace analysis:
  - Output is written as fp16 (harness gate is rel_err < 2e-2; fp16
    rounding contributes ~5e-4).  Halves the dominant HBM write traffic
    to ~28.9 MB/core.  Host converts back to f32 (free).
  - Weights are block-diagonal-packed ON HOST into wpack[9, 128, 2048]
    fp16 (wpack[kk][(gc,a), oct*512+gc*64+f] = matrix[kk*32+oct*8+gc, a,
    f]); uploaded as a plain contiguous input.  This deletes v1's whole
    memset/paint/cast weight build (which serialized the prologue to
    ~55 us before the first output DMA).  The 9 x 512 KB loads go on the
    GPSIMD SWDGE ring so both HWDGE rings stay free for x + outputs.
  - x is cast f32->fp16 before the PE transposes (fp32 transposes stream
    at 1/4 rate), and matmuls run fp16 x fp16 -> f32 PSUM.
  - Weights-stationary matmuls: stationary = wpack chunk [K=128, M=128
    f-cols], moving = tap [K=128, N=784 positions].  vs v1's
    tap-stationary form this cuts streamed PE columns 147K -> 113K and
    makes every output M=128 wide.  Output becomes f-major
    o[kk, f=2048, pos=784]; the host untangles (free).
  - Per (kk, oct, chunk-pair): 2 matmuls into [128,784] PSUM tiles,
    PSUM->SBUF fp16 copies split DVE/ACT, one 401 KB contiguous DMA
    alternating the two HWDGE rings.
  - Tap compaction (im2col gather) per tap kk>=1: octs 0-1 on GPSIMD,
    oct 2 on DVE, oct 3 on ACT; tap 0 per-batch on DVE/ACT right after
    each batch's transposes so the first matmul fires ~10 us in.
"""

import numpy as np

B, H, W, C, A = 4, 16, 16, 32, 16
KS = 3
OH = OW = 14
NCAP = KS * KS * C          # 288 capsules
FTOT = 512                  # filter*atom
NCORES = 8
FPC = FTOT // NCORES        # 64 output features per core
POS = B * OH * OW           # 784 output positions

_NC_CACHE = {}


def _build_nc():
    import concourse.bass as bass  # noqa: F401
    import concourse.mybir as mybir
    import concourse.tile as tile
    from concourse import bacc, masks

    f32 = mybir.dt.float32
    f16 = mybir.dt.float16
    bf16 = mybir.dt.bfloat16

    nc = bacc.Bacc(None, target_bir_lowering=False)
    x_d = nc.declare_dram_parameter("x", [B, H, W, C, A], f32, isOutput=False)
    w_d = nc.declare_dram_parameter("wpack", [KS * KS, 128, 4 * 512], bf16,
                                    isOutput=False)
    # f-major output: o[kk, f(oct*512+gc*64+f64), pos].  Each inner DMA
    # writes one fully contiguous 401 KB block; host untangles kk/f.
    o_d = nc.declare_dram_parameter("out", [KS * KS, 2048, POS], f16,
                                    isOutput=True)

    x2d = x_d.rearrange("b h w c a -> (b h w) (c a)")   # [1024, 512]
    # pair-of-chunks view: [9, 8, 128, 2, 784]; one DMA writes 2 chunks
    # (two 1568 B runs per partition, 401 KB total, fully contiguous).
    ov = o_d.rearrange("k (g h p) q -> k g p h q", p=128, h=2)

    with tile.TileContext(nc) as tc:
        with (
            tc.tile_pool(name="const", bufs=1) as constp,
            tc.tile_pool(name="big", bufs=1) as bigp,
            tc.tile_pool(name="stage", bufs=4) as stagep,
            tc.tile_pool(name="tapp", bufs=2) as tapp,
            tc.tile_pool(name="psumtr", bufs=2, space="PSUM") as psumtr,
            tc.tile_pool(name="psummm", bufs=4, space="PSUM") as psummm,
        ):
            ident = constp.tile([128, 128], bf16, tag="ident")
            masks.make_identity(nc, ident[:])

            # ---- weights: 9 contiguous 512 KB fp16 loads on SWDGE ----
            wps = [
                bigp.tile([128, 4 * 512], bf16, tag=f"wp{kk}", name=f"wp{kk}")
                for kk in range(9)
            ]
            for kk in range(9):
                nc.gpsimd.dma_start(wps[kk][:], w_d[kk])

            # ---- x: HBM -> SBUF, four [128, 2*512] f32 tiles ----
            # tile t = batch t (rows t*256..t*256+255 of x2d).
            xsrc = x2d.rearrange("(t s p) c -> t p s c", t=4, p=128)
            x_sbs = [
                bigp.tile([128, 2 * 512], f32, tag=f"x_sb{t}", name=f"x_sb{t}")
                for t in range(4)
            ]
            for t in range(4):
                eng = nc.sync if t % 2 == 0 else nc.scalar
                eng.dma_start(
                    x_sbs[t][:].rearrange("p (s c) -> p s c", s=2), xsrc[t]
                )

            # ---- cast x to fp16, then PE-transpose into per-octet tiles
            # xt[oct][(dc,a), (b,h,w)] ----
            x16s = [
                bigp.tile([128, 2 * 512], bf16, tag=f"x16_{t}", name=f"x16_{t}")
                for t in range(4)
            ]
            xts = [
                bigp.tile([128, 1024], bf16, tag=f"xt{o}", name=f"xt{o}")
                for o in range(4)
            ]
            xtvs = [
                t[:].rearrange("p (b h w) -> p b h w", b=B, h=H) for t in xts
            ]
            tap0 = tapp.tile([128, 4 * POS], bf16, tag="tap")
            t0v = [
                tap0[:, o * POS:(o + 1) * POS].rearrange(
                    "p (b i j) -> p b i j", b=B, i=OH
                )
                for o in range(4)
            ]
            for t in range(4):
                if t % 2 == 0:
                    nc.vector.tensor_copy(x16s[t][:], x_sbs[t][:])
                else:
                    nc.scalar.copy(x16s[t][:], x_sbs[t][:])
                for s in (2 * t, 2 * t + 1):
                    for oct in range(4):
                        tr = psumtr.tile([128, 128], bf16, tag="tr")
                        nc.tensor.transpose(
                            tr[:],
                            x16s[t][
                                :, (s % 2) * 512 + oct * 128:
                                (s % 2) * 512 + (oct + 1) * 128
                            ],
                            ident[:],
                        )
                        dst = xts[oct][:, s * 128:(s + 1) * 128]
                        if (s + oct) % 2 == 0:
                            nc.vector.tensor_copy(dst, tr[:])
                        else:
                            nc.scalar.copy(dst, tr[:])
                # batch t of tap 0 compacts as soon as its transposes land
                for oct in range(4):
                    src = xtvs[oct][:, t:t + 1, 0:OH, 0:OW]
                    if (t + oct) % 2 == 0:
                        nc.vector.tensor_copy(t0v[oct][:, t:t + 1], src)
                    else:
                        nc.scalar.copy(t0v[oct][:, t:t + 1], src)

            # ---- main loop: 9 taps x 4 octs x 2 chunk-pairs ----
            it = 0
            for kk in range(9):
                ki, kj = kk // 3, kk % 3
                if kk == 0:
                    tap = tap0
                else:
                    tap = tapp.tile([128, 4 * POS], bf16, tag="tap")
                    for oct in range(4):
                        dst = tap[:, oct * POS:(oct + 1) * POS].rearrange(
                            "p (b i j) -> p b i j", b=B, i=OH
                        )
                        src = xtvs[oct][:, :, ki: ki + OH, kj: kj + OW]
                        if oct == 2:
                            nc.vector.tensor_copy(dst, src)
                        else:
                            nc.gpsimd.tensor_copy(dst, src)
                for oct in range(4):
                    for c2 in range(2):
                        st = stagep.tile([128, 2 * POS], f16, tag="st")
                        for h2 in range(2):
                            ch = c2 * 2 + h2
                            wchunk = wps[kk][
                                :, oct * 512 + ch * 128:
                                oct * 512 + (ch + 1) * 128
                            ]
                            # PSUM bank limit: 512 f32/partition, so each
                            # chunk runs as two N=392 matmuls; the fp16
                            # PSUM->SBUF casts then split DVE | ACT.
                            for q in range(2):
                                ps = psummm.tile([128, 392], f32, tag="mm")
                                nc.tensor.matmul(
                                    ps[:],
                                    wchunk,
                                    tap[:, oct * POS + q * 392:
                                        oct * POS + (q + 1) * 392],
                                    start=True,
                                    stop=True,
                                )
                                dst = st[:, h2 * POS + q * 392:
                                         h2 * POS + (q + 1) * 392]
                                if q == 0:
                                    nc.vector.tensor_copy(dst, ps[:])
                                else:
                                    nc.scalar.copy(dst, ps[:])
                        dma_eng = nc.sync if it % 2 == 0 else nc.scalar
                        dma_eng.dma_start(
                            ov[kk, oct * 2 + c2],
                            st[:].rearrange("p (h q) -> p h q", h=2),
                        )
                        it += 1

    nc.compile()
    return nc


def _get_nc():
    if "nc" not in _NC_CACHE:
        _NC_CACHE["nc"] = _build_nc()
    return _NC_CACHE["nc"]


def _pack_weights(matrix):
    """matrix [288,16,512] f32 -> per-core block-diag wpack [8][9,128,2048]
    fp16.  wpack[c][kk, gc*16+a, oct*512+gc*64+f] =
    matrix[kk*32+oct*8+gc, a, c*64+f]."""
    m = matrix.reshape(KS * KS, 4, 8, A, NCORES, FPC)  # [kk,oct,gc,a,core,f]
    import ml_dtypes
    out = np.zeros((NCORES, KS * KS, 128, 2048), dtype=ml_dtypes.bfloat16)
    for gc in range(8):
        # rows gc*16..gc*16+16, cols oct*512+gc*64..+64
        blk = m[:, :, gc].astype(ml_dtypes.bfloat16)    # [kk,oct,a,core,f]
        for oct in range(4):
            out[:, :, gc * A:(gc + 1) * A,
                oct * 512 + gc * FPC: oct * 512 + (gc + 1) * FPC] = (
                blk[:, oct].transpose(2, 0, 1, 3)      # [core,kk,a,f]
            )
    return out


def _core_inputs(x, matrix):
    x = np.ascontiguousarray(x, dtype=np.float32)
    wp = _pack_weights(np.asarray(matrix, dtype=np.float32))
    return [
        {"x": x, "wpack": np.ascontiguousarray(wp[c])}
        for c in range(NCORES)
    ]


def _unscramble(parts):
    """parts: [8][9, 2048, 784] fp16 -> [4,14,14,288,32,16] f32."""
    arr = np.stack(parts)                              # [core,kk,col,pos]
    arr = arr.reshape(NCORES, KS * KS, 4, 8, FPC, POS)
    arr = arr.transpose(5, 1, 2, 3, 0, 4)              # [pos,kk,oct,gc,core,f]
    full = arr.reshape(POS, NCAP, FTOT).astype(np.float32)
    return np.ascontiguousarray(
        full.reshape(B, OH, OW, NCAP, 32, 16)
    )


def kernel(x, matrix):
    from concourse.bass_utils import run_bass_kernel_spmd

    nc = _get_nc()
    in_maps = _core_inputs(x, matrix)
    r = run_bass_kernel_spmd(nc, in_maps, list(range(NCORES)))
    return _unscramble([r.results[c]["out"] for c in range(NCORES)])


# revision 14
# speedup vs baseline: 1.2450x; 1.2450x over previous
"""CapsuleTransformConv on 8 Trainium2 NeuronCores (bf16/fp16 pipeline).

Problem:  x [4,16,16,32,16] f32, matrix [288,16,512] f32.
          im2col (K=3, VALID) -> tile [4,14,14,288,16]
          votes  = einsum('bhwna,nac->bhwnc', tile, matrix)
          out    = votes.reshape(4,14,14,288,32,16)

Sharding: tensor-parallel over the filter*atom output axis (512 -> 64 per
core).  Every core reads the full x and its 64-wide weight slice; writes
its 784 x 288 x 64 output slice (fp16, ~29 MB -- the dominant traffic).

Design (v4), driven by trace analysis of earlier revisions:
  - Output is fp16 (harness gate is rel_err < 2e-2; measured ~3e-3 total
    with bf16 matmul inputs).  Host converts back to f32 (free).
  - x and the weights are cast/packed ON HOST: x uploads as bf16 (1 MB),
    weights upload as the block-diagonal wpack[9, 128, 2048] bf16
    (wpack[kk][(gc,a), oct*512+gc*64+f] = matrix[kk*32+oct*8+gc, a, f]).
    No on-chip weight build at all; the 9 x 512 KB wpack loads ride the
    otherwise-idle GPSIMD SWDGE ring.
  - Weights-stationary matmuls: stationary = wpack chunk [K=128, M=128
    f-cols], moving = tap positions.  Every output is M=128 wide and the
    output is f-major o[kk, f=2048, pos=784] (host untangles).  PSUM
    rules (out <= 512 f32/partition, bank-aligned) make each chunk two
    matmuls, N=512 + N=272, into one [128,784] 2-bank PSUM tile.
  - One whole-chunk PSUM->SBUF fp16 cast per chunk (fixed per-op cost
    ~230 ns makes split copies wasteful), alternating DVE / ACT;
    two chunks stage into a [128, 2*784] fp16 tile and leave in one
    401 KB contiguous DMA issued from the Sync ring (HWDGE), keeping
    both copy engines free of DMA-issue work.
  - Tap compaction (im2col gather): tap 0 per-batch on DVE/ACT right
    after each batch's transposes (first matmul ~10 us in); taps >= 1
    on GPSIMD (octs 0-1) / DVE (oct 2) / ACT (oct 3), prefetched a tap
    ahead via double-buffered tap tiles.
"""

import numpy as np

B, H, W, C, A = 4, 16, 16, 32, 16
KS = 3
OH = OW = 14
NCAP = KS * KS * C          # 288 capsules
FTOT = 512                  # filter*atom
NCORES = 8
FPC = FTOT // NCORES        # 64 output features per core
POS = B * OH * OW           # 784 output positions

_NC_CACHE = {}


def _build_nc():
    import concourse.bass as bass  # noqa: F401
    import concourse.mybir as mybir
    import concourse.tile as tile
    from concourse import bacc, masks

    f16 = mybir.dt.float16
    f32 = mybir.dt.float32
    bf16 = mybir.dt.bfloat16

    nc = bacc.Bacc(None, target_bir_lowering=False)
    x_d = nc.declare_dram_parameter("x", [B * H * W, C * A], bf16,
                                    isOutput=False)
    w_d = nc.declare_dram_parameter("wpack", [KS * KS, 128, 4 * 512], bf16,
                                    isOutput=False)
    # f-major output: o[kk, f(oct*512+gc*64+f64), pos].
    o_d = nc.declare_dram_parameter("out", [KS * KS, 2048, POS], f16,
                                    isOutput=True)

    # pair-of-chunks view [9, 8, 128, 2, 784]: one DMA per 2 chunks.
    ov = o_d.rearrange("k (g h p) q -> k g p h q", p=128, h=2)

    with tile.TileContext(nc) as tc:
        with (
            tc.tile_pool(name="const", bufs=1) as constp,
            tc.tile_pool(name="big", bufs=1) as bigp,
            tc.tile_pool(name="stage", bufs=4) as stagep,
            tc.tile_pool(name="tapp", bufs=2) as tapp,
            tc.tile_pool(name="psumtr", bufs=2, space="PSUM") as psumtr,
            tc.tile_pool(name="psummm", bufs=3, space="PSUM") as psummm,
        ):
            ident = constp.tile([128, 128], bf16, tag="ident")
            masks.make_identity(nc, ident[:])

            # ---- weights: 9 contiguous 512 KB bf16 loads on SWDGE ----
            wps = [
                bigp.tile([128, 4 * 512], bf16, tag=f"wp{kk}", name=f"wp{kk}")
                for kk in range(9)
            ]
            for kk in range(9):
                nc.gpsimd.dma_start(wps[kk][:], w_d[kk])

            # ---- x (bf16): four [128, 2*512] tiles; tile t = batch t ----
            xsrc = x_d.rearrange("(t s p) c -> t p s c", t=4, p=128)
            x16s = [
                bigp.tile([128, 2 * 512], bf16, tag=f"x16_{t}", name=f"x16_{t}")
                for t in range(4)
            ]
            for t in range(4):
                eng = nc.sync if t % 2 == 0 else nc.scalar
                eng.dma_start(
                    x16s[t][:].rearrange("p (s c) -> p s c", s=2), xsrc[t]
                )

            # ---- PE-transpose into per-octet xt[oct][(dc,a), (b,h,w)] ----
            xts = [
                bigp.tile([128, 1024], bf16, tag=f"xt{o}", name=f"xt{o}")
                for o in range(4)
            ]
            xtvs = [
                t[:].rearrange("p (b h w) -> p b h w", b=B, h=H) for t in xts
            ]
            tap0 = tapp.tile([128, 4 * POS], bf16, tag="tap")
            t0v = [
                tap0[:, o * POS:(o + 1) * POS].rearrange(
                    "p (b i j) -> p b i j", b=B, i=OH
                )
                for o in range(4)
            ]
            for t in range(4):
                for s in (2 * t, 2 * t + 1):
                    for oct in range(4):
                        tr = psumtr.tile([128, 128], bf16, tag="tr")
                        nc.tensor.transpose(
                            tr[:],
                            x16s[t][
                                :, (s % 2) * 512 + oct * 128:
                                (s % 2) * 512 + (oct + 1) * 128
                            ],
                            ident[:],
                        )
                        dst = xts[oct][:, s * 128:(s + 1) * 128]
                        if (s + oct) % 2 == 0:
                            nc.vector.tensor_copy(dst, tr[:])
                        else:
                            nc.scalar.copy(dst, tr[:])
                # batch t of tap 0 compacts as soon as its transposes land
                for oct in range(4):
                    src = xtvs[oct][:, t:t + 1, 0:OH, 0:OW]
                    if (t + oct) % 2 == 0:
                        nc.vector.tensor_copy(t0v[oct][:, t:t + 1], src)
                    else:
                        nc.scalar.copy(t0v[oct][:, t:t + 1], src)

            # ---- main loop: 9 taps x 4 octs x 4 chunks ----
            it = 0
            for kk in range(9):
                ki, kj = kk // 3, kk % 3
                if kk == 0:
                    tap = tap0
                else:
                    tap = tapp.tile([128, 4 * POS], bf16, tag="tap")
                    for oct in range(4):
                        dst = tap[:, oct * POS:(oct + 1) * POS].rearrange(
                            "p (b i j) -> p b i j", b=B, i=OH
                        )
                        src = xtvs[oct][:, :, ki: ki + OH, kj: kj + OW]
                        if oct < 2:
                            nc.gpsimd.tensor_copy(dst, src)
                        elif oct == 2:
                            nc.vector.tensor_copy(dst, src)
                        else:
                            nc.scalar.copy(dst, src)
                for oct in range(4):
                    for c2 in range(2):
                        st = stagep.tile([128, 2 * POS], f16, tag="st")
                        for h2 in range(2):
                            ch = c2 * 2 + h2
                            wchunk = wps[kk][
                                :, oct * 512 + ch * 128:
                                oct * 512 + (ch + 1) * 128
                            ]
                            # one [128,784] f32 PSUM tile per chunk, filled
                            # by two bank-aligned matmuls (512 | 272)
                            ps = psummm.tile([128, POS], f32, tag="mm")
                            for q0, qn in ((0, 512), (512, 272)):
                                nc.tensor.matmul(
                                    ps[:, q0:q0 + qn],
                                    wchunk,
                                    tap[:, oct * POS + q0:
                                        oct * POS + q0 + qn],
                                    start=True,
                                    stop=True,
                                )
                            # whole-chunk fp16 cast, alternating DVE | ACT
                            dst = st[:, h2 * POS:(h2 + 1) * POS]
                            if it % 2 == 0:
                                nc.vector.tensor_copy(dst, ps[:])
                            else:
                                nc.scalar.copy(dst, ps[:])
                            it += 1
                        nc.sync.dma_start(
                            ov[kk, oct * 2 + c2],
                            st[:].rearrange("p (h q) -> p h q", h=2),
                        )

    nc.compile()
    return nc


def _get_nc():
    if "nc" not in _NC_CACHE:
        _NC_CACHE["nc"] = _build_nc()
    return _NC_CACHE["nc"]


def _pack_weights(matrix):
    """matrix [288,16,512] f32 -> per-core block-diag wpack [8][9,128,2048]
    bf16.  wpack[c][kk, gc*16+a, oct*512+gc*64+f] =
    matrix[kk*32+oct*8+gc, a, c*64+f]."""
    import ml_dtypes
    m = matrix.reshape(KS * KS, 4, 8, A, NCORES, FPC)  # [kk,oct,gc,a,core,f]
    out = np.zeros((NCORES, KS * KS, 128, 2048), dtype=ml_dtypes.bfloat16)
    for gc in range(8):
        blk = m[:, :, gc].astype(ml_dtypes.bfloat16)   # [kk,oct,a,core,f]
        for oct in range(4):
            out[:, :, gc * A:(gc + 1) * A,
                oct * 512 + gc * FPC: oct * 512 + (gc + 1) * FPC] = (
                blk[:, oct].transpose(2, 0, 1, 3)      # [core,kk,a,f]
            )
    return out


def _core_inputs(x, matrix):
    import ml_dtypes
    xb = np.ascontiguousarray(
        np.asarray(x, dtype=np.float32).reshape(B * H * W, C * A)
    ).astype(ml_dtypes.bfloat16)
    wp = _pack_weights(np.asarray(matrix, dtype=np.float32))
    return [
        {"x": xb, "wpack": np.ascontiguousarray(wp[c])}
        for c in range(NCORES)
    ]


def _unscramble(parts):
    """parts: [8][9, 2048, 784] fp16 -> [4,14,14,288,32,16] f32."""
    arr = np.stack(parts)                              # [core,kk,col,pos]
    arr = arr.reshape(NCORES, KS * KS, 4, 8, FPC, POS)
    arr = arr.transpose(5, 1, 2, 3, 0, 4)              # [pos,kk,oct,gc,core,f]
    full = arr.reshape(POS, NCAP, FTOT).astype(np.float32)
    return np.ascontiguousarray(
        full.reshape(B, OH, OW, NCAP, 32, 16)
    )


def kernel(x, matrix):
    from concourse.bass_utils import run_bass_kernel_spmd

    nc = _get_nc()
    in_maps = _core_inputs(x, matrix)
    r = run_bass_kernel_spmd(nc, in_maps, list(range(NCORES)))
    return _unscramble([r.results[c]["out"] for c in range(NCORES)])


# revision 15
# speedup vs baseline: 1.3183x; 1.0589x over previous
"""CapsuleTransformConv on 8 Trainium2 NeuronCores (bf16/fp16 pipeline).

Problem:  x [4,16,16,32,16] f32, matrix [288,16,512] f32.
          im2col (K=3, VALID) -> tile [4,14,14,288,16]
          votes  = einsum('bhwna,nac->bhwnc', tile, matrix)
          out    = votes.reshape(4,14,14,288,32,16)

Sharding: tensor-parallel over the filter*atom output axis (512 -> 64 per
core).  Every core reads the full x and its 64-wide weight slice; writes
its 784 x 288 x 64 output slice (fp16, ~29 MB -- the dominant traffic).

Design (v4), driven by trace analysis of earlier revisions:
  - Output is fp16 (harness gate is rel_err < 2e-2; measured ~3e-3 total
    with bf16 matmul inputs).  Host converts back to f32 (free).
  - x and the weights are cast/packed ON HOST: x uploads as bf16 (1 MB),
    weights upload as the block-diagonal wpack[9, 128, 2048] bf16
    (wpack[kk][(gc,a), oct*512+gc*64+f] = matrix[kk*32+oct*8+gc, a, f]).
    No on-chip weight build at all; the 9 x 512 KB wpack loads ride the
    otherwise-idle GPSIMD SWDGE ring.
  - Weights-stationary matmuls: stationary = wpack chunk [K=128, M=128
    f-cols], moving = tap positions.  Every output is M=128 wide and the
    output is f-major o[kk, f=2048, pos=784] (host untangles).  PSUM
    rules (out <= 512 f32/partition, bank-aligned) make each chunk two
    matmuls, N=512 + N=272, into one [128,784] 2-bank PSUM tile.
  - One whole-chunk PSUM->SBUF fp16 cast per chunk (fixed per-op cost
    ~230 ns makes split copies wasteful), alternating DVE / ACT;
    two chunks stage into a [128, 2*784] fp16 tile and leave in one
    401 KB contiguous DMA issued from the Sync ring (HWDGE), keeping
    both copy engines free of DMA-issue work.
  - Tap compaction (im2col gather): tap 0 per-batch on DVE/ACT right
    after each batch's transposes (first matmul ~10 us in); taps >= 1
    on GPSIMD (octs 0-1) / DVE (oct 2) / ACT (oct 3), prefetched a tap
    ahead via double-buffered tap tiles.
"""

import numpy as np

B, H, W, C, A = 4, 16, 16, 32, 16
KS = 3
OH = OW = 14
NCAP = KS * KS * C          # 288 capsules
FTOT = 512                  # filter*atom
NCORES = 8
FPC = FTOT // NCORES        # 64 output features per core
POS = B * OH * OW           # 784 output positions

_NC_CACHE = {}


def _build_nc():
    import concourse.bass as bass  # noqa: F401
    import concourse.mybir as mybir
    import concourse.tile as tile
    from concourse import bacc, masks

    f16 = mybir.dt.float16
    f32 = mybir.dt.float32
    bf16 = mybir.dt.bfloat16

    nc = bacc.Bacc(None, target_bir_lowering=False)
    x_d = nc.declare_dram_parameter("x", [B * H * W, C * A], bf16,
                                    isOutput=False)
    w_d = nc.declare_dram_parameter("wpack", [KS * KS, 128, 4 * 512], bf16,
                                    isOutput=False)
    # f-major output: o[kk, f(oct*512+gc*64+f64), pos].
    o_d = nc.declare_dram_parameter("out", [KS * KS, 2048, POS], f16,
                                    isOutput=True)

    # pair-of-chunks view [9, 8, 128, 2, 784]: one DMA per 2 chunks.
    ov = o_d.rearrange("k (g h p) q -> k g p h q", p=128, h=2)

    with tile.TileContext(nc) as tc:
        with (
            tc.tile_pool(name="const", bufs=1) as constp,
            tc.tile_pool(name="big", bufs=1) as bigp,
            tc.tile_pool(name="stage", bufs=4) as stagep,
            tc.tile_pool(name="tapp", bufs=2) as tapp,
            tc.tile_pool(name="psumtr", bufs=2, space="PSUM") as psumtr,
            tc.tile_pool(name="psummm", bufs=6, space="PSUM") as psummm,
        ):
            ident = constp.tile([128, 128], bf16, tag="ident")
            masks.make_identity(nc, ident[:])

            # ---- weights: 9 contiguous 512 KB bf16 loads on SWDGE ----
            wps = [
                bigp.tile([128, 4 * 512], bf16, tag=f"wp{kk}", name=f"wp{kk}")
                for kk in range(9)
            ]
            for kk in range(3):
                nc.gpsimd.dma_start(wps[kk][:], w_d[kk])

            # ---- x (bf16): four [128, 2*512] tiles; tile t = batch t ----
            xsrc = x_d.rearrange("(t s p) c -> t p s c", t=4, p=128)
            x16s = [
                bigp.tile([128, 2 * 512], bf16, tag=f"x16_{t}", name=f"x16_{t}")
                for t in range(4)
            ]
            for t in range(4):
                eng = nc.sync if t % 2 == 0 else nc.scalar
                eng.dma_start(
                    x16s[t][:].rearrange("p (s c) -> p s c", s=2), xsrc[t]
                )

            # ---- PE-transpose into per-octet xt[oct][(dc,a), (b,h,w)] ----
            xts = [
                bigp.tile([128, 1024], bf16, tag=f"xt{o}", name=f"xt{o}")
                for o in range(4)
            ]
            xtvs = [
                t[:].rearrange("p (b h w) -> p b h w", b=B, h=H) for t in xts
            ]
            tap0 = tapp.tile([128, 4 * POS], bf16, tag="tap")
            t0v = [
                tap0[:, o * POS:(o + 1) * POS].rearrange(
                    "p (b i j) -> p b i j", b=B, i=OH
                )
                for o in range(4)
            ]
            for t in range(4):
                for s in (2 * t, 2 * t + 1):
                    for oct in range(4):
                        tr = psumtr.tile([128, 128], bf16, tag="tr")
                        nc.tensor.transpose(
                            tr[:],
                            x16s[t][
                                :, (s % 2) * 512 + oct * 128:
                                (s % 2) * 512 + (oct + 1) * 128
                            ],
                            ident[:],
                        )
                        dst = xts[oct][:, s * 128:(s + 1) * 128]
                        if (s + oct) % 2 == 0:
                            nc.vector.tensor_copy(dst, tr[:])
                        else:
                            nc.scalar.copy(dst, tr[:])
                # batch t of tap 0 compacts as soon as its transposes land
                for oct in range(4):
                    src = xtvs[oct][:, t:t + 1, 0:OH, 0:OW]
                    if (t + oct) % 2 == 0:
                        nc.vector.tensor_copy(t0v[oct][:, t:t + 1], src)
                    else:
                        nc.scalar.copy(t0v[oct][:, t:t + 1], src)

            # ---- main loop: 9 taps x 4 octs x 4 chunks ----
            it = 0
            for kk in range(9):
                ki, kj = kk // 3, kk % 3
                if kk == 0:
                    tap = tap0
                else:
                    tap = tapp.tile([128, 4 * POS], bf16, tag="tap")
                    for oct in range(4):
                        dst = tap[:, oct * POS:(oct + 1) * POS].rearrange(
                            "p (b i j) -> p b i j", b=B, i=OH
                        )
                        src = xtvs[oct][:, :, ki: ki + OH, kj: kj + OW]
                        if oct < 3:
                            nc.gpsimd.tensor_copy(dst, src)
                        else:
                            nc.scalar.copy(dst, src)
                    if kk + 2 < 9:
                        nc.gpsimd.dma_start(wps[kk + 2][:], w_d[kk + 2])
                for oct in range(4):
                    for c2 in range(2):
                        st = stagep.tile([128, 2 * POS], f16, tag="st")
                        for h2 in range(2):
                            ch = c2 * 2 + h2
                            wchunk = wps[kk][
                                :, oct * 512 + ch * 128:
                                oct * 512 + (ch + 1) * 128
                            ]
                            # two single-bank PSUM tiles per chunk
                            # (512 | 272); deep pool keeps PE matmuls
                            # back-to-back.  fp16 casts alternate the
                            # big/small halves across DVE | ACT.
                            for q0, qn in ((0, 512), (512, 272)):
                                ps = psummm.tile([128, qn], f32, tag="mm")
                                nc.tensor.matmul(
                                    ps[:],
                                    wchunk,
                                    tap[:, oct * POS + q0:
                                        oct * POS + q0 + qn],
                                    start=True,
                                    stop=True,
                                )
                                dst = st[:, h2 * POS + q0:
                                         h2 * POS + q0 + qn]
                                if (it + (0 if qn == 512 else 1)) % 2 == 0:
                                    nc.vector.tensor_copy(dst, ps[:])
                                else:
                                    nc.scalar.copy(dst, ps[:])
                            it += 1
                        nc.sync.dma_start(
                            ov[kk, oct * 2 + c2],
                            st[:].rearrange("p (h q) -> p h q", h=2),
                        )

    nc.compile()
    return nc


def _get_nc():
    if "nc" not in _NC_CACHE:
        _NC_CACHE["nc"] = _build_nc()
    return _NC_CACHE["nc"]


def _pack_weights(matrix):
    """matrix [288,16,512] f32 -> per-core block-diag wpack [8][9,128,2048]
    bf16.  wpack[c][kk, gc*16+a, oct*512+gc*64+f] =
    matrix[kk*32+oct*8+gc, a, c*64+f]."""
    import ml_dtypes
    m = matrix.reshape(KS * KS, 4, 8, A, NCORES, FPC)  # [kk,oct,gc,a,core,f]
    out = np.zeros((NCORES, KS * KS, 128, 2048), dtype=ml_dtypes.bfloat16)
    for gc in range(8):
        blk = m[:, :, gc].astype(ml_dtypes.bfloat16)   # [kk,oct,a,core,f]
        for oct in range(4):
            out[:, :, gc * A:(gc + 1) * A,
                oct * 512 + gc * FPC: oct * 512 + (gc + 1) * FPC] = (
                blk[:, oct].transpose(2, 0, 1, 3)      # [core,kk,a,f]
            )
    return out


def _core_inputs(x, matrix):
    import ml_dtypes
    xb = np.ascontiguousarray(
        np.asarray(x, dtype=np.float32).reshape(B * H * W, C * A)
    ).astype(ml_dtypes.bfloat16)
    wp = _pack_weights(np.asarray(matrix, dtype=np.float32))
    return [
        {"x": xb, "wpack": np.ascontiguousarray(wp[c])}
        for c in range(NCORES)
    ]


def _unscramble(parts):
    """parts: [8][9, 2048, 784] fp16 -> [4,14,14,288,32,16] f32."""
    arr = np.stack(parts)                              # [core,kk,col,pos]
    arr = arr.reshape(NCORES, KS * KS, 4, 8, FPC, POS)
    arr = arr.transpose(5, 1, 2, 3, 0, 4)              # [pos,kk,oct,gc,core,f]
    full = arr.reshape(POS, NCAP, FTOT).astype(np.float32)
    return np.ascontiguousarray(
        full.reshape(B, OH, OW, NCAP, 32, 16)
    )


def kernel(x, matrix):
    from concourse.bass_utils import run_bass_kernel_spmd

    nc = _get_nc()
    in_maps = _core_inputs(x, matrix)
    r = run_bass_kernel_spmd(nc, in_maps, list(range(NCORES)))
    return _unscramble([r.results[c]["out"] for c in range(NCORES)])
